# revision 8
# baseline (speedup 1.0000x reference)
"""Trainium2 Bass kernel for gated attention (nn_Attention_57475252355505).

Reference computation (per batch b):
    q = (q_x @ Wq.T) * 1/sqrt(32), split into H=8 heads of D=32
    k = kv_x @ Wk.T ; v = kv_x @ Wv.T
    a = softmax(q @ k.T + bias)           # bias broadcast over heads
    o = (a @ v) * sigmoid(q_x @ Wg.T + bg)
    out = o @ Wo.T + bo

Sharding: 8 cores, core c handles batch b = c//4 and query rows
[512*(c%4), 512*(c%4+1)).  kv_x/weights are replicated per batch group;
bias/q_x/output are disjoint.  No collectives needed.

Dataflow on each core is in "transposed space" ([feature, token] layouts)
so that every matmul contraction sits on the partition axis:
  - scores are computed as S^T [k, q] so softmax-over-k can use the
    matmul ones-trick for denominators, and the o-matmul needs no
    transposition of the (huge) attention-weight matrix.
  - bias^T is produced once with TensorE identity-matmuls and injected
    into the scores PSUM accumulation (so no elementwise bias pass).
  - the D=32 contractions are packed 4-per-PE-array with tile_position.
"""

import sys

sys.path.insert(0, "/opt/trn_rl_repo")

import numpy as np

import concourse.bass as bass
import concourse.mybir as mybir
import concourse.tile as tile_mod
from concourse.bass_utils import run_bass_kernel_spmd

# ---------------------------------------------------------------------------
# Problem constants (hardcoded per the harness contract).
B, Q, K, C, H, D = 2, 2048, 2048, 256, 8, 32
N_CORES = 8
QS = Q * B // N_CORES  # 512 query rows per core
SCALE = 1.0 / np.sqrt(np.float32(D))

FP32 = mybir.dt.float32
BF16 = mybir.dt.bfloat16

# ---------------------------------------------------------------------------
# This walrus build only accepts a single sync-wait per instruction; Tile's
# semaphore assignment batches several.  After tracing, hoist extra waits
# onto single-wait NOPs on the same engine (same blocking semantics).


def _split_multi_waits(nc):
    for fn in nc.m.functions:
        for bb in fn.blocks:
            insts = bb.instructions
            new = []
            changed = False
            for inst in insts:
                si = inst.sync_info
                if si is not None and len(si.on_wait) > 1:
                    changed = True
                    waits = list(si.on_wait)
                    for w in waits[:-1]:
                        nop = mybir.InstNoOp(
                            name=f"I-wsplit-{nc.next_id()}", ins=[], outs=[]
                        )
                        nop.engine = inst.engine
                        nop.sync_info = mybir.SyncInfo(on_wait=[w], on_update=[])
                        nc.register_instruction(nop)
                        new.append(nop)
                    inst.sync_info = mybir.SyncInfo(
                        on_wait=[waits[-1]], on_update=list(si.on_update)
                    )
                new.append(inst)
            if changed:
                bb.instructions = new


# ---------------------------------------------------------------------------


def _fill_identity(nc, ident_ap, fill):
    """ident[x, y] = fill if x == y else 0."""
    nc.gpsimd.memset(ident_ap, 0.0)
    nc.gpsimd.affine_select(
        out=ident_ap,
        in_=ident_ap,
        compare_op=mybir.AluOpType.not_equal,
        fill=fill,
        base=0,
        pattern=[[-1, ident_ap.shape[1]]],
        channel_multiplier=1,
    )


def build_graph():
    """Build the per-core Bass graph (same graph SPMD on all 8 cores)."""
    nc = bass.Bass()

    # --- DRAM parameters (per-core shards; names must match in_maps keys) ---
    p_qx = nc.declare_dram_parameter("q_x", [QS, C], FP32, isOutput=False)
    p_kvx = nc.declare_dram_parameter("kv_x", [K, C], FP32, isOutput=False)
    p_bias = nc.declare_dram_parameter("bias", [QS, K], FP32, isOutput=False)
    p_wq = nc.declare_dram_parameter("Wq", [C, C], FP32, isOutput=False)
    p_wk = nc.declare_dram_parameter("Wk", [C, C], FP32, isOutput=False)
    p_wv = nc.declare_dram_parameter("Wv", [C, C], FP32, isOutput=False)
    p_wo = nc.declare_dram_parameter("Wo", [C, C], FP32, isOutput=False)
    p_bo = nc.declare_dram_parameter("bo", [C], FP32, isOutput=False)
    p_wg = nc.declare_dram_parameter("Wg", [C, C], FP32, isOutput=False)
    p_bg = nc.declare_dram_parameter("bg", [C], FP32, isOutput=False)
    p_out = nc.declare_dram_parameter("out", [QS, C], FP32, isOutput=True)

    NKT = K // 128  # 16 key tiles
    NCT = C // 128  # 2 feature tiles
    NQT = QS // 128  # 4 query sub-tiles

    with tile_mod.TileContext(nc) as tc:
        with (
            tc.tile_pool(name="const", bufs=1) as constp,
            tc.tile_pool(name="persist", bufs=1) as persist,
        ):
            # ---- constants ----
            ident = constp.tile([128, 128], BF16, tag="ident")
            _fill_identity(nc, ident[:], 1.0)
            ident_s = constp.tile([128, 128], BF16, tag="ident_s")
            _fill_identity(nc, ident_s[:], float(SCALE))
            ones_mat = constp.tile([128, 32], BF16, tag="ones_mat")
            nc.gpsimd.memset(ones_mat[:], 1.0)
            ones_row = constp.tile([1, 128], BF16, tag="ones_row")
            nc.gpsimd.memset(ones_row[:], 1.0)

            bg_half = constp.tile([128, NCT], FP32, tag="bg_half")
            nc.gpsimd.dma_start(
                bg_half[:], p_bg[:].rearrange("(ct p) -> p ct", p=128)
            )
            nc.vector.tensor_scalar_mul(bg_half[:], bg_half[:], 0.5)
            bo_row = constp.tile([1, C], BF16, tag="bo_row")
            nc.gpsimd.dma_start(bo_row[:], p_bo[:].rearrange("(a c) -> a c", a=1))

            wt = {}
            qxT, kvT, kT, qT = [], [], [], []
            g_half = []

            with (
                tc.tile_pool(name="stage", bufs=1) as stage,
                tc.tile_pool(name="evp", bufs=4, space="PSUM") as evp,
            ):
                # ---- load + transpose the five weight matrices (bf16) ----
                # wt[w][ct] : [128, C], partition = input channel c (within
                # tile ct), free = output channel j.
                for name, par, scaled in (
                    ("Wq", p_wq, True),
                    ("Wk", p_wk, False),
                    ("Wv", p_wv, False),
                    ("Wg", p_wg, False),
                    ("Wo", p_wo, False),
                ):
                    w_nat = stage.tile([128, NCT, C], BF16, tag="w_nat", bufs=2)
                    nc.gpsimd.dma_start(
                        w_nat[:], par[:].rearrange("(jt p) c -> p jt c", p=128)
                    )
                    tiles = []
                    for ct in range(NCT):
                        ps = evp.tile([128, 512], FP32, tag="ev")
                        for jt in range(NCT):
                            nc.tensor.matmul(
                                ps[:, jt * 128 : (jt + 1) * 128],
                                w_nat[:, jt, ct * 128 : (ct + 1) * 128],
                                ident_s[:] if scaled else ident[:],
                                start=True,
                                stop=True,
                            )
                        sb = persist.tile([128, C], BF16, tag=f"wt_{name}_{ct}")
                        nc.vector.tensor_copy(sb[:], ps[:, :C])
                        tiles.append(sb)
                    wt[name] = tiles

                # ---- load + transpose activations ----
                qx_nat = stage.tile([128, NQT, C], BF16, tag="qx_nat")
                nc.gpsimd.dma_start(
                    qx_nat[:], p_qx[:].rearrange("(qt p) c -> p qt c", p=128)
                )
                for ct in range(NCT):
                    ps = evp.tile([128, 512], FP32, tag="ev")
                    for qt in range(NQT):
                        nc.tensor.matmul(
                            ps[:, qt * 128 : (qt + 1) * 128],
                            qx_nat[:, qt, ct * 128 : (ct + 1) * 128],
                            ident[:],
                            start=True,
                            stop=True,
                        )
                    sb = persist.tile([128, QS], BF16, tag=f"qxT_{ct}")
                    nc.vector.tensor_copy(sb[:], ps[:])
                    qxT.append(sb)

                kv_nat = stage.tile([128, NKT, C], BF16, tag="kv_nat")
                nc.gpsimd.dma_start(
                    kv_nat[:], p_kvx[:].rearrange("(kt p) c -> p kt c", p=128)
                )
                for ct in range(NCT):
                    sb = persist.tile([128, K], BF16, tag=f"kvT_{ct}")
                    for g in range(NKT // 4):  # 4 transposes -> one evac
                        ps = evp.tile([128, 512], FP32, tag="ev")
                        for i in range(4):
                            kt = g * 4 + i
                            nc.tensor.matmul(
                                ps[:, i * 128 : (i + 1) * 128],
                                kv_nat[:, kt, ct * 128 : (ct + 1) * 128],
                                ident[:],
                                start=True,
                                stop=True,
                            )
                        nc.vector.tensor_copy(
                            sb[:, g * 512 : (g + 1) * 512], ps[:]
                        )
                    kvT.append(sb)

                # ---- bias^T (bf16, via identity-matmul transposes) ----
                bias_nat = stage.tile([128, NQT, K], BF16, tag="bias_nat")
                nc.gpsimd.dma_start(
                    bias_nat[:], p_bias[:].rearrange("(qt p) k -> p qt k", p=128)
                )
                biasT = persist.tile([128, NKT, QS], BF16, tag="biasT")
                for kt in range(NKT):
                    ps = evp.tile([128, 512], FP32, tag="ev")
                    for qt in range(NQT):
                        nc.tensor.matmul(
                            ps[:, qt * 128 : (qt + 1) * 128],
                            bias_nat[:, qt, kt * 128 : (kt + 1) * 128],
                            ident[:],
                            start=True,
                            stop=True,
                        )
                    nc.vector.tensor_copy(biasT[:, kt, :], ps[:])

                # ---- projections ----
                # kT[jt]: [128, K]  (partition j = (head % 4) * 32 + d)
                for jt in range(NCT):
                    sb = persist.tile([128, K], BF16, tag=f"kT_{jt}")
                    for tc_ in range(K // 512):
                        ps = evp.tile([128, 512], FP32, tag="ev")
                        for ct in range(NCT):
                            nc.tensor.matmul(
                                ps[:],
                                wt["Wk"][ct][:, jt * 128 : (jt + 1) * 128],
                                kvT[ct][:, tc_ * 512 : (tc_ + 1) * 512],
                                start=(ct == 0),
                                stop=(ct == NCT - 1),
                            )
                        nc.vector.tensor_copy(
                            sb[:, tc_ * 512 : (tc_ + 1) * 512], ps[:]
                        )
                    kT.append(sb)

                # qT[jt]: [128, QS] (pre-scaled by 1/sqrt(D) via ident_s)
                for jt in range(NCT):
                    ps = evp.tile([128, 512], FP32, tag="ev")
                    for ct in range(NCT):
                        nc.tensor.matmul(
                            ps[:],
                            wt["Wq"][ct][:, jt * 128 : (jt + 1) * 128],
                            qxT[ct][:],
                            start=(ct == 0),
                            stop=(ct == NCT - 1),
                        )
                    sb = persist.tile([128, QS], BF16, tag=f"qT_{jt}")
                    nc.vector.tensor_copy(sb[:], ps[:])
                    qT.append(sb)

                # gate: tanh(0.5*x + 0.5*bg); sigmoid(x+bg) = 0.5*tanh + 0.5
                for jt in range(NCT):
                    ps = evp.tile([128, 512], FP32, tag="ev")
                    for ct in range(NCT):
                        nc.tensor.matmul(
                            ps[:],
                            wt["Wg"][ct][:, jt * 128 : (jt + 1) * 128],
                            qxT[ct][:],
                            start=(ct == 0),
                            stop=(ct == NCT - 1),
                        )
                    th = persist.tile([128, QS], BF16, tag=f"gtanh_{jt}")
                    nc.scalar.activation(
                        th[:],
                        ps[:],
                        mybir.ActivationFunctionType.Tanh,
                        bias=bg_half[:, jt : jt + 1],
                        scale=0.5,
                    )
                    g_half.append(th)

                # v[kt]: [128, C] natural layout (partition = key token)
                v_sb = persist.tile([128, NKT, C], BF16, tag="v_sb")
                for kt in range(NKT):
                    ps = evp.tile([128, 512], FP32, tag="ev")
                    for ct in range(NCT):
                        nc.tensor.matmul(
                            ps[:, :C],
                            kvT[ct][:, kt * 128 : (kt + 1) * 128],
                            wt["Wv"][ct][:],
                            start=(ct == 0),
                            stop=(ct == NCT - 1),
                        )
                    nc.vector.tensor_copy(v_sb[:, kt, :], ps[:, :C])

            # ---- attention core ----
            ogT = []
            with (
                tc.tile_pool(name="acc", bufs=1, space="PSUM") as accp,
                tc.tile_pool(name="scores", bufs=1, space="PSUM") as scoresp,
                tc.tile_pool(name="expp", bufs=3) as expp,
            ):
                oT_ps = [
                    accp.tile([128, QS], FP32, tag=f"oT_{w}", name=f"oT_{w}") for w in range(2)
                ]
                sums_ps = [
                    accp.tile([128, QS], FP32, tag=f"sums_{w}", name=f"sums_{w}") for w in range(2)
                ]

                for kt in range(NKT):
                    for w in range(2):  # head wave: heads 4w .. 4w+3
                        sc = scoresp.tile([128, 4 * QS], FP32, tag="sc")
                        for s in range(4):
                            nc.tensor.matmul(
                                sc[:, s * QS : (s + 1) * QS],
                                ident[:],
                                biasT[:, kt, :],
                                start=True,
                                stop=False,
                            )
                            nc.tensor.matmul(
                                sc[:, s * QS : (s + 1) * QS],
                                kT[w][
                                    32 * s : 32 * (s + 1),
                                    kt * 128 : (kt + 1) * 128,
                                ],
                                qT[w][32 * s : 32 * (s + 1), :],
                                start=False,
                                stop=True,
                                tile_position=(32 * s, 0),
                            )
                        ex = expp.tile([128, 4 * QS], BF16, tag="ex")
                        nc.scalar.activation(
                            ex[:], sc[:], mybir.ActivationFunctionType.Exp
                        )
                        first, last = kt == 0, kt == NKT - 1
                        for s in range(4):
                            nc.tensor.matmul(
                                oT_ps[w][32 * s : 32 * (s + 1), :],
                                v_sb[:, kt, (4 * w + s) * D : (4 * w + s + 1) * D],
                                ex[:, s * QS : (s + 1) * QS],
                                start=first,
                                stop=last,
                                tile_position=(0, 32 * s),
                            )
                            nc.tensor.matmul(
                                sums_ps[w][32 * s : 32 * (s + 1), :],
                                ones_mat[:],
                                ex[:, s * QS : (s + 1) * QS],
                                start=first,
                                stop=last,
                                tile_position=(0, 32 * s),
                            )

                # ---- normalize + gate:  og = oT * g * (1/Z) ----
                # sums_ps rows 32s..32s+32 all hold head (4w+s)'s Z[q].
                recipz = persist.tile([128, 2, QS], FP32, tag="recipz")
                for w in range(2):
                    nc.vector.reciprocal(recipz[:, w, :], sums_ps[w][:])
                for w in range(2):
                    zg = persist.tile([128, QS], BF16, tag=f"zg_{w}")
                    nc.vector.tensor_scalar(
                        zg[:],
                        g_half[w][:],
                        0.5,
                        0.5,
                        mybir.AluOpType.mult,
                        mybir.AluOpType.add,
                    )
                    nc.vector.tensor_mul(zg[:], zg[:], recipz[:, w, :])
                    og = persist.tile([128, QS], BF16, tag=f"ogT_{w}")
                    nc.vector.tensor_mul(og[:], oT_ps[w][:], zg[:])
                    ogT.append(og)

            # ---- output projection (natural layout) + bo ----
            out_sb = persist.tile([128, NQT, C], FP32, tag="out_sb")
            with tc.tile_pool(name="outp", bufs=2, space="PSUM") as outp:
                for qt in range(NQT):
                    ps = outp.tile([128, C], FP32, tag="outps")
                    for ct in range(NCT):
                        nc.tensor.matmul(
                            ps[:],
                            ogT[ct][:, qt * 128 : (qt + 1) * 128],
                            wt["Wo"][ct][:],
                            start=(ct == 0),
                            stop=False,
                        )
                    nc.tensor.matmul(
                        ps[:],
                        ones_row[:],
                        bo_row[:],
                        start=False,
                        stop=True,
                    )
                    nc.vector.tensor_copy(out_sb[:, qt, :], ps[:])

            nc.sync.dma_start(
                p_out[:].rearrange("(qt p) c -> p qt c", p=128), out_sb[:]
            )

    _split_multi_waits(nc)
    return nc


# ---------------------------------------------------------------------------


def _shard_inputs(inputs):
    """Full inputs -> per-core input maps."""
    in_maps = []
    for c in range(N_CORES):
        b, qc = divmod(c, 4)
        qs = qc * QS
        m = {
            "q_x": inputs["q_x"][b, qs : qs + QS, :],
            "kv_x": inputs["kv_x"][b],
            "bias": inputs["bias"][b, 0, qs : qs + QS, :],
            "Wq": inputs["Wq"],
            "Wk": inputs["Wk"],
            "Wv": inputs["Wv"],
            "Wo": inputs["Wo"],
            "bo": inputs["bo"],
            "Wg": inputs["Wg"],
            "bg": inputs["bg"],
        }
        m = {
            k: np.ascontiguousarray(np.asarray(v, dtype=np.float32))
            for k, v in m.items()
        }
        in_maps.append(m)
    return in_maps


def run(inputs, trace=False, tmpdir=None):
    """Run the kernel; returns (full_output, BassKernelResults)."""
    nc = build_graph()
    in_maps = _shard_inputs(inputs)
    res = run_bass_kernel_spmd(
        nc, in_maps, core_ids=list(range(N_CORES)), trace=trace, tmpdir=tmpdir
    )
    out = np.empty((B, Q, C), dtype=np.float32)
    for c in range(N_CORES):
        b, qc = divmod(c, 4)
        out[b, qc * QS : (qc + 1) * QS, :] = res.results[c]["out"]
    return out, res


def kernel(**inputs):
    out, _ = run(inputs, trace=False)
    return out


# revision 9
# speedup vs baseline: 1.1938x; 1.1938x over previous
"""Trainium2 Bass kernel for gated attention (nn_Attention_57475252355505).

Reference computation (per batch b):
    q = (q_x @ Wq.T) * 1/sqrt(32), split into H=8 heads of D=32
    k = kv_x @ Wk.T ; v = kv_x @ Wv.T
    a = softmax(q @ k.T + bias)           # bias broadcast over heads
    o = (a @ v) * sigmoid(q_x @ Wg.T + bg)
    out = o @ Wo.T + bo

Sharding: 8 cores, core c handles batch b = c//4 and query rows
[512*(c%4), 512*(c%4+1)).  kv_x/weights are replicated per batch group;
bias/q_x/output are disjoint.  No collectives needed.

Dataflow on each core is in "transposed space" ([feature, token] layouts)
so that every matmul contraction sits on the partition axis:
  - scores are computed as S^T [k, q] so softmax-over-k can use the
    matmul ones-trick for denominators, and the o-matmul needs no
    transposition of the (huge) attention-weight matrix.
  - bias^T is produced once with TensorE identity-matmuls and injected
    into the scores PSUM accumulation (so no elementwise bias pass).
  - the D=32 contractions are packed 4-per-PE-array with tile_position.
"""

import sys

sys.path.insert(0, "/opt/trn_rl_repo")

import numpy as np

import concourse.bass as bass
import concourse.mybir as mybir
import concourse.tile as tile_mod
from concourse.bass_utils import run_bass_kernel_spmd

# ---------------------------------------------------------------------------
# Problem constants (hardcoded per the harness contract).
B, Q, K, C, H, D = 2, 2048, 2048, 256, 8, 32
N_CORES = 8
QS = Q * B // N_CORES  # 512 query rows per core
SCALE = 1.0 / np.sqrt(np.float32(D))

FP32 = mybir.dt.float32
BF16 = mybir.dt.bfloat16

# ---------------------------------------------------------------------------
# This walrus build only accepts a single sync-wait per instruction; Tile's
# semaphore assignment batches several.  After tracing, hoist extra waits
# onto single-wait NOPs on the same engine (same blocking semantics).


def _split_multi_waits(nc):
    for fn in nc.m.functions:
        for bb in fn.blocks:
            insts = bb.instructions
            new = []
            changed = False
            for inst in insts:
                si = inst.sync_info
                if si is not None and len(si.on_wait) > 1:
                    changed = True
                    waits = list(si.on_wait)
                    for w in waits[:-1]:
                        nop = mybir.InstNoOp(
                            name=f"I-wsplit-{nc.next_id()}", ins=[], outs=[]
                        )
                        nop.engine = inst.engine
                        nop.sync_info = mybir.SyncInfo(on_wait=[w], on_update=[])
                        nc.register_instruction(nop)
                        new.append(nop)
                    inst.sync_info = mybir.SyncInfo(
                        on_wait=[waits[-1]], on_update=list(si.on_update)
                    )
                new.append(inst)
            if changed:
                bb.instructions = new


# ---------------------------------------------------------------------------


def _fill_identity(nc, ident_ap, fill):
    """ident[x, y] = fill if x == y else 0."""
    nc.gpsimd.memset(ident_ap, 0.0)
    nc.gpsimd.affine_select(
        out=ident_ap,
        in_=ident_ap,
        compare_op=mybir.AluOpType.not_equal,
        fill=fill,
        base=0,
        pattern=[[-1, ident_ap.shape[1]]],
        channel_multiplier=1,
    )


def build_graph():
    """Build the per-core Bass graph (same graph SPMD on all 8 cores)."""
    nc = bass.Bass()

    # --- DRAM parameters (per-core shards; names must match in_maps keys) ---
    p_qx = nc.declare_dram_parameter("q_x", [QS, C], FP32, isOutput=False)
    p_kvx = nc.declare_dram_parameter("kv_x", [K, C], FP32, isOutput=False)
    p_bias = nc.declare_dram_parameter("bias", [QS, K], FP32, isOutput=False)
    p_wq = nc.declare_dram_parameter("Wq", [C, C], FP32, isOutput=False)
    p_wk = nc.declare_dram_parameter("Wk", [C, C], FP32, isOutput=False)
    p_wv = nc.declare_dram_parameter("Wv", [C, C], FP32, isOutput=False)
    p_wo = nc.declare_dram_parameter("Wo", [C, C], FP32, isOutput=False)
    p_bo = nc.declare_dram_parameter("bo", [C], FP32, isOutput=False)
    p_wg = nc.declare_dram_parameter("Wg", [C, C], FP32, isOutput=False)
    p_bg = nc.declare_dram_parameter("bg", [C], FP32, isOutput=False)
    p_out = nc.declare_dram_parameter("out", [QS, C], FP32, isOutput=True)

    NKT = K // 128  # 16 key tiles
    NCT = C // 128  # 2 feature tiles
    NQT = QS // 128  # 4 query sub-tiles

    with tile_mod.TileContext(nc) as tc:
        with (
            tc.tile_pool(name="const", bufs=1) as constp,
            tc.tile_pool(name="persist", bufs=1) as persist,
        ):
            # ---- constants ----
            ident = constp.tile([128, 128], BF16, tag="ident")
            _fill_identity(nc, ident[:], 1.0)
            ident_s = constp.tile([128, 128], BF16, tag="ident_s")
            _fill_identity(nc, ident_s[:], float(SCALE))
            ones_mat = constp.tile([128, 32], BF16, tag="ones_mat")
            nc.gpsimd.memset(ones_mat[:], 1.0)
            ones_row = constp.tile([1, 128], BF16, tag="ones_row")
            nc.gpsimd.memset(ones_row[:], 1.0)

            bg_half = constp.tile([128, NCT], FP32, tag="bg_half")
            nc.gpsimd.dma_start(
                bg_half[:], p_bg[:].rearrange("(ct p) -> p ct", p=128)
            )
            nc.vector.tensor_scalar_mul(bg_half[:], bg_half[:], 0.5)
            bo_row = constp.tile([1, C], BF16, tag="bo_row")
            nc.gpsimd.dma_start(bo_row[:], p_bo[:].rearrange("(a c) -> a c", a=1))

            wt = {}
            qxT, kvT, kT, qT = [], [], [], []
            g_half = []

            with (
                tc.tile_pool(name="stage", bufs=1) as stage,
                tc.tile_pool(name="evp", bufs=4, space="PSUM") as evp,
            ):
                # ---- load + transpose the five weight matrices (bf16) ----
                # wt[w][ct] : [128, C], partition = input channel c (within
                # tile ct), free = output channel j.
                for name, par, scaled in (
                    ("Wq", p_wq, True),
                    ("Wk", p_wk, False),
                    ("Wv", p_wv, False),
                    ("Wg", p_wg, False),
                    ("Wo", p_wo, False),
                ):
                    w_nat = stage.tile([128, NCT, C], BF16, tag="w_nat", bufs=2)
                    nc.gpsimd.dma_start(
                        w_nat[:], par[:].rearrange("(jt p) c -> p jt c", p=128)
                    )
                    tiles = []
                    for ct in range(NCT):
                        ps = evp.tile([128, 512], FP32, tag="ev")
                        for jt in range(NCT):
                            nc.tensor.matmul(
                                ps[:, jt * 128 : (jt + 1) * 128],
                                w_nat[:, jt, ct * 128 : (ct + 1) * 128],
                                ident_s[:] if scaled else ident[:],
                                start=True,
                                stop=True,
                            )
                        sb = persist.tile([128, C], BF16, tag=f"wt_{name}_{ct}")
                        nc.vector.tensor_copy(sb[:], ps[:, :C])
                        tiles.append(sb)
                    wt[name] = tiles

                # ---- load + transpose activations ----
                qx_nat = stage.tile([128, NQT, C], BF16, tag="qx_nat")
                nc.gpsimd.dma_start(
                    qx_nat[:], p_qx[:].rearrange("(qt p) c -> p qt c", p=128)
                )
                for ct in range(NCT):
                    ps = evp.tile([128, 512], FP32, tag="ev")
                    for qt in range(NQT):
                        nc.tensor.matmul(
                            ps[:, qt * 128 : (qt + 1) * 128],
                            qx_nat[:, qt, ct * 128 : (ct + 1) * 128],
                            ident[:],
                            start=True,
                            stop=True,
                        )
                    sb = persist.tile([128, QS], BF16, tag=f"qxT_{ct}")
                    nc.vector.tensor_copy(sb[:], ps[:])
                    qxT.append(sb)

                kv_nat = stage.tile([128, NKT, C], BF16, tag="kv_nat")
                nc.gpsimd.dma_start(
                    kv_nat[:], p_kvx[:].rearrange("(kt p) c -> p kt c", p=128)
                )
                for ct in range(NCT):
                    sb = persist.tile([128, K], BF16, tag=f"kvT_{ct}")
                    for g in range(NKT // 4):  # 4 transposes -> one evac
                        ps = evp.tile([128, 512], FP32, tag="ev")
                        for i in range(4):
                            kt = g * 4 + i
                            nc.tensor.matmul(
                                ps[:, i * 128 : (i + 1) * 128],
                                kv_nat[:, kt, ct * 128 : (ct + 1) * 128],
                                ident[:],
                                start=True,
                                stop=True,
                            )
                        nc.vector.tensor_copy(
                            sb[:, g * 512 : (g + 1) * 512], ps[:]
                        )
                    kvT.append(sb)

                # ---- bias^T (bf16, via identity-matmul transposes) ----
                bias_nat = stage.tile([128, NQT, K], BF16, tag="bias_nat")
                nc.gpsimd.dma_start(
                    bias_nat[:], p_bias[:].rearrange("(qt p) k -> p qt k", p=128)
                )
                expBT = persist.tile([128, NKT, QS], BF16, tag="expBT")
                for kt in range(NKT):
                    ps = evp.tile([128, 512], FP32, tag="ev")
                    for qt in range(NQT):
                        nc.tensor.matmul(
                            ps[:, qt * 128 : (qt + 1) * 128],
                            bias_nat[:, qt, kt * 128 : (kt + 1) * 128],
                            ident[:],
                            start=True,
                            stop=True,
                        )
                    nc.scalar.activation(
                        expBT[:, kt, :], ps[:], mybir.ActivationFunctionType.Exp
                    )

                # ---- projections ----
                # kT[jt]: [128, K]  (partition j = (head % 4) * 32 + d)
                for jt in range(NCT):
                    sb = persist.tile([128, K], BF16, tag=f"kT_{jt}")
                    for tc_ in range(K // 512):
                        ps = evp.tile([128, 512], FP32, tag="ev")
                        for ct in range(NCT):
                            nc.tensor.matmul(
                                ps[:],
                                wt["Wk"][ct][:, jt * 128 : (jt + 1) * 128],
                                kvT[ct][:, tc_ * 512 : (tc_ + 1) * 512],
                                start=(ct == 0),
                                stop=(ct == NCT - 1),
                            )
                        nc.scalar.copy(sb[:, tc_ * 512 : (tc_ + 1) * 512], ps[:])
                    kT.append(sb)

                # qT[jt]: [128, QS] (pre-scaled by 1/sqrt(D) via ident_s)
                for jt in range(NCT):
                    ps = evp.tile([128, 512], FP32, tag="ev")
                    for ct in range(NCT):
                        nc.tensor.matmul(
                            ps[:],
                            wt["Wq"][ct][:, jt * 128 : (jt + 1) * 128],
                            qxT[ct][:],
                            start=(ct == 0),
                            stop=(ct == NCT - 1),
                        )
                    sb = persist.tile([128, QS], BF16, tag=f"qT_{jt}")
                    nc.scalar.copy(sb[:], ps[:])
                    qT.append(sb)

                # gate: tanh(0.5*x + 0.5*bg); sigmoid(x+bg) = 0.5*tanh + 0.5
                for jt in range(NCT):
                    ps = evp.tile([128, 512], FP32, tag="ev")
                    for ct in range(NCT):
                        nc.tensor.matmul(
                            ps[:],
                            wt["Wg"][ct][:, jt * 128 : (jt + 1) * 128],
                            qxT[ct][:],
                            start=(ct == 0),
                            stop=(ct == NCT - 1),
                        )
                    th = persist.tile([128, QS], BF16, tag=f"gtanh_{jt}")
                    nc.scalar.activation(
                        th[:],
                        ps[:],
                        mybir.ActivationFunctionType.Tanh,
                        bias=bg_half[:, jt : jt + 1],
                        scale=0.5,
                    )
                    g_half.append(th)

                # v[kt]: [128, C] natural layout (partition = key token)
                v_sb = persist.tile([128, NKT, C], BF16, tag="v_sb")
                for kt in range(NKT):
                    ps = evp.tile([128, 512], FP32, tag="ev")
                    for ct in range(NCT):
                        nc.tensor.matmul(
                            ps[:, :C],
                            kvT[ct][:, kt * 128 : (kt + 1) * 128],
                            wt["Wv"][ct][:],
                            start=(ct == 0),
                            stop=(ct == NCT - 1),
                        )
                    nc.scalar.copy(v_sb[:, kt, :], ps[:, :C])

            # ---- attention core ----
            ogT = []
            with (
                tc.tile_pool(name="acc", bufs=1, space="PSUM") as accp,
                tc.tile_pool(name="scores", bufs=1, space="PSUM") as scoresp,
                tc.tile_pool(name="expp", bufs=3) as expp,
            ):
                oT_ps = [
                    accp.tile([128, QS], FP32, tag=f"oT_{w}", name=f"oT_{w}") for w in range(2)
                ]
                sums_ps = [
                    accp.tile([128, QS], FP32, tag=f"sums_{w}", name=f"sums_{w}") for w in range(2)
                ]

                for kt in range(NKT):
                    for w in range(2):  # head wave: heads 4w .. 4w+3
                        sc = scoresp.tile([128, 4 * QS], FP32, tag="sc")
                        # 16-tile K=32/M=32 grid: row = head, col = k-subtile
                        # (diagonal emission so LDWEIGHTS can pull ahead)
                        for dd in range(4):
                            for s in range(4):
                                ks = (s + dd) % 4
                                nc.tensor.matmul(
                                    sc[32 * ks : 32 * (ks + 1), s * QS : (s + 1) * QS],
                                    kT[w][
                                        32 * s : 32 * (s + 1),
                                        kt * 128 + 32 * ks : kt * 128 + 32 * (ks + 1),
                                    ],
                                    qT[w][32 * s : 32 * (s + 1), :],
                                    start=True,
                                    stop=True,
                                    tile_position=(32 * s, 32 * ks),
                                )
                        ex = expp.tile([128, 4 * QS], BF16, tag="ex")
                        nc.scalar.activation(
                            ex[:], sc[:], mybir.ActivationFunctionType.Exp
                        )
                        at = expp.tile([128, 4, QS], BF16, tag="at")
                        nc.vector.tensor_tensor(
                            out=at[:],
                            in0=ex[:].rearrange("p (h q) -> p h q", h=4),
                            in1=expBT[:, kt, :].unsqueeze(1).broadcast_to(
                                (128, 4, QS)
                            ),
                            op=mybir.AluOpType.mult,
                        )
                        first, last = kt == 0, kt == NKT - 1
                        for s in range(4):
                            nc.tensor.matmul(
                                oT_ps[w][32 * s : 32 * (s + 1), :],
                                v_sb[:, kt, (4 * w + s) * D : (4 * w + s + 1) * D],
                                at[:, s, :],
                                start=first,
                                stop=last,
                                tile_position=(0, 32 * s),
                            )
                        for s in range(4):
                            nc.tensor.matmul(
                                sums_ps[w][32 * s : 32 * (s + 1), :],
                                ones_mat[:],
                                at[:, s, :],
                                start=first,
                                stop=last,
                                tile_position=(0, 32 * s),
                            )

                # ---- normalize + gate:  og = oT * g * (1/Z) ----
                # sums_ps rows 32s..32s+32 all hold head (4w+s)'s Z[q].
                recipz = persist.tile([128, 2, QS], FP32, tag="recipz")
                for w in range(2):
                    nc.vector.reciprocal(recipz[:, w, :], sums_ps[w][:])
                for w in range(2):
                    zg = persist.tile([128, QS], BF16, tag=f"zg_{w}")
                    nc.vector.tensor_scalar(
                        zg[:],
                        g_half[w][:],
                        0.5,
                        0.5,
                        mybir.AluOpType.mult,
                        mybir.AluOpType.add,
                    )
                    nc.vector.tensor_mul(zg[:], zg[:], recipz[:, w, :])
                    og = persist.tile([128, QS], BF16, tag=f"ogT_{w}")
                    nc.vector.tensor_mul(og[:], oT_ps[w][:], zg[:])
                    ogT.append(og)

            # ---- output projection (natural layout) + bo ----
            out_sb = persist.tile([128, NQT, C], FP32, tag="out_sb")
            with tc.tile_pool(name="outp", bufs=2, space="PSUM") as outp:
                for qt in range(NQT):
                    ps = outp.tile([128, C], FP32, tag="outps")
                    for ct in range(NCT):
                        nc.tensor.matmul(
                            ps[:],
                            ogT[ct][:, qt * 128 : (qt + 1) * 128],
                            wt["Wo"][ct][:],
                            start=(ct == 0),
                            stop=False,
                        )
                    nc.tensor.matmul(
                        ps[:],
                        ones_row[:],
                        bo_row[:],
                        start=False,
                        stop=True,
                    )
                    nc.vector.tensor_copy(out_sb[:, qt, :], ps[:])

            nc.sync.dma_start(
                p_out[:].rearrange("(qt p) c -> p qt c", p=128), out_sb[:]
            )

    _split_multi_waits(nc)
    return nc


# ---------------------------------------------------------------------------


def _shard_inputs(inputs):
    """Full inputs -> per-core input maps."""
    in_maps = []
    for c in range(N_CORES):
        b, qc = divmod(c, 4)
        qs = qc * QS
        m = {
            "q_x": inputs["q_x"][b, qs : qs + QS, :],
            "kv_x": inputs["kv_x"][b],
            "bias": inputs["bias"][b, 0, qs : qs + QS, :],
            "Wq": inputs["Wq"],
            "Wk": inputs["Wk"],
            "Wv": inputs["Wv"],
            "Wo": inputs["Wo"],
            "bo": inputs["bo"],
            "Wg": inputs["Wg"],
            "bg": inputs["bg"],
        }
        m = {
            k: np.ascontiguousarray(np.asarray(v, dtype=np.float32))
            for k, v in m.items()
        }
        in_maps.append(m)
    return in_maps


def run(inputs, trace=False, tmpdir=None):
    """Run the kernel; returns (full_output, BassKernelResults)."""
    nc = build_graph()
    in_maps = _shard_inputs(inputs)
    res = run_bass_kernel_spmd(
        nc, in_maps, core_ids=list(range(N_CORES)), trace=trace, tmpdir=tmpdir
    )
    out = np.empty((B, Q, C), dtype=np.float32)
    for c in range(N_CORES):
        b, qc = divmod(c, 4)
        out[b, qc * QS : (qc + 1) * QS, :] = res.results[c]["out"]
    return out, res


def kernel(**inputs):
    out, _ = run(inputs, trace=False)
    return out


# revision 10
# speedup vs baseline: 1.6331x; 1.3679x over previous
"""Trainium2 Bass kernel for gated attention (nn_Attention_57475252355505).

Reference computation (per batch b):
    q = (q_x @ Wq.T) * 1/sqrt(32), split into H=8 heads of D=32
    k = kv_x @ Wk.T ; v = kv_x @ Wv.T
    a = softmax(q @ k.T + bias)           # bias broadcast over heads
    o = (a @ v) * sigmoid(q_x @ Wg.T + bg)
    out = o @ Wo.T + bo

Sharding: 8 cores, core c handles batch b = c//4 and query rows
[512*(c%4), 512*(c%4+1)).  kv_x/weights are replicated per batch group;
bias/q_x/output are disjoint.  No collectives needed.

Dataflow on each core is in "transposed space" ([feature, token] layouts)
so that every matmul contraction sits on the partition axis:
  - scores are computed as S^T [k, q] so softmax-over-k can use the
    matmul ones-trick for denominators, and the o-matmul needs no
    transposition of the (huge) attention-weight matrix.
  - bias^T is produced once with TensorE identity-matmuls and injected
    into the scores PSUM accumulation (so no elementwise bias pass).
  - the D=32 contractions are packed 4-per-PE-array with tile_position.
"""

import sys

sys.path.insert(0, "/opt/trn_rl_repo")

import numpy as np

import concourse.bass as bass
import concourse.mybir as mybir
import concourse.tile as tile_mod
from concourse.bass_utils import run_bass_kernel_spmd

# ---------------------------------------------------------------------------
# Problem constants (hardcoded per the harness contract).
B, Q, K, C, H, D = 2, 2048, 2048, 256, 8, 32
N_CORES = 8
QS = Q * B // N_CORES  # 512 query rows per core
SCALE = 1.0 / np.sqrt(np.float32(D))

FP32 = mybir.dt.float32
BF16 = mybir.dt.bfloat16

# ---------------------------------------------------------------------------
# This walrus build only accepts a single sync-wait per instruction; Tile's
# semaphore assignment batches several.  After tracing, hoist extra waits
# onto single-wait NOPs on the same engine (same blocking semantics).


def _split_multi_waits(nc):
    for fn in nc.m.functions:
        for bb in fn.blocks:
            insts = bb.instructions
            new = []
            changed = False
            for inst in insts:
                si = inst.sync_info
                if si is not None and len(si.on_wait) > 1:
                    changed = True
                    waits = list(si.on_wait)
                    for w in waits[:-1]:
                        nop = mybir.InstNoOp(
                            name=f"I-wsplit-{nc.next_id()}", ins=[], outs=[]
                        )
                        nop.engine = inst.engine
                        nop.sync_info = mybir.SyncInfo(on_wait=[w], on_update=[])
                        nc.register_instruction(nop)
                        new.append(nop)
                    inst.sync_info = mybir.SyncInfo(
                        on_wait=[waits[-1]], on_update=list(si.on_update)
                    )
                new.append(inst)
            if changed:
                bb.instructions = new


# ---------------------------------------------------------------------------


def _fill_identity(nc, ident_ap, fill):
    """ident[x, y] = fill if x == y else 0."""
    nc.gpsimd.memset(ident_ap, 0.0)
    nc.gpsimd.affine_select(
        out=ident_ap,
        in_=ident_ap,
        compare_op=mybir.AluOpType.not_equal,
        fill=fill,
        base=0,
        pattern=[[-1, ident_ap.shape[1]]],
        channel_multiplier=1,
    )


def build_graph():
    """Build the per-core Bass graph (same graph SPMD on all 8 cores)."""
    nc = bass.Bass()

    # --- DRAM parameters (per-core shards; names must match in_maps keys) ---
    p_qx = nc.declare_dram_parameter("q_x", [QS, C], FP32, isOutput=False)
    p_kvx = nc.declare_dram_parameter("kv_x", [K, C], FP32, isOutput=False)
    p_bias = nc.declare_dram_parameter("bias", [QS, K], FP32, isOutput=False)
    p_wq = nc.declare_dram_parameter("Wq", [C, C], FP32, isOutput=False)
    p_wk = nc.declare_dram_parameter("Wk", [C, C], FP32, isOutput=False)
    p_wv = nc.declare_dram_parameter("Wv", [C, C], FP32, isOutput=False)
    p_wo = nc.declare_dram_parameter("Wo", [C, C], FP32, isOutput=False)
    p_bo = nc.declare_dram_parameter("bo", [C], FP32, isOutput=False)
    p_wg = nc.declare_dram_parameter("Wg", [C, C], FP32, isOutput=False)
    p_bg = nc.declare_dram_parameter("bg", [C], FP32, isOutput=False)
    p_out = nc.declare_dram_parameter("out", [QS, C], FP32, isOutput=True)

    NKT = K // 128  # 16 key tiles
    NCT = C // 128  # 2 feature tiles
    NQT = QS // 128  # 4 query sub-tiles

    with tile_mod.TileContext(nc) as tc:
        with (
            tc.tile_pool(name="const", bufs=1) as constp,
            tc.tile_pool(name="persist", bufs=1) as persist,
        ):
            # ---- constants ----
            ident = constp.tile([128, 128], BF16, tag="ident")
            _fill_identity(nc, ident[:], 1.0)
            ident_s = constp.tile([128, 128], BF16, tag="ident_s")
            _fill_identity(nc, ident_s[:], float(SCALE))
            ones_mat = constp.tile([128, 32], BF16, tag="ones_mat")
            nc.gpsimd.memset(ones_mat[:], 1.0)
            ones_row = constp.tile([1, 128], BF16, tag="ones_row")
            nc.gpsimd.memset(ones_row[:], 1.0)

            bg_half = constp.tile([128, NCT], FP32, tag="bg_half")
            nc.gpsimd.dma_start(
                bg_half[:], p_bg[:].rearrange("(ct p) -> p ct", p=128)
            )
            nc.vector.tensor_scalar_mul(bg_half[:], bg_half[:], 0.5)
            bo_row = constp.tile([1, C], BF16, tag="bo_row")
            nc.gpsimd.dma_start(bo_row[:], p_bo[:].rearrange("(a c) -> a c", a=1))

            wt = {}
            qxT, kvT, kT, qT = [], [], [], []
            g_half = []

            with (
                tc.tile_pool(name="stage", bufs=1) as stage,
                tc.tile_pool(name="evp", bufs=4, space="PSUM") as evp,
            ):
                # ---- load + transpose the five weight matrices (bf16) ----
                # wt[w][ct] : [128, C], partition = input channel c (within
                # tile ct), free = output channel j.
                for name, par, scaled in (
                    ("Wq", p_wq, True),
                    ("Wk", p_wk, False),
                    ("Wv", p_wv, False),
                    ("Wg", p_wg, False),
                    ("Wo", p_wo, False),
                ):
                    w_nat = stage.tile([128, NCT, C], BF16, tag="w_nat", bufs=2)
                    nc.gpsimd.dma_start(
                        w_nat[:], par[:].rearrange("(jt p) c -> p jt c", p=128)
                    )
                    tiles = []
                    for ct in range(NCT):
                        ps = evp.tile([128, 512], FP32, tag="ev")
                        for jt in range(NCT):
                            nc.tensor.matmul(
                                ps[:, jt * 128 : (jt + 1) * 128],
                                w_nat[:, jt, ct * 128 : (ct + 1) * 128],
                                ident_s[:] if scaled else ident[:],
                                start=True,
                                stop=True,
                            )
                        sb = persist.tile([128, C], BF16, tag=f"wt_{name}_{ct}")
                        nc.vector.tensor_copy(sb[:], ps[:, :C])
                        tiles.append(sb)
                    wt[name] = tiles

                # ---- load + transpose activations ----
                qx_nat = stage.tile([128, NQT, C], BF16, tag="qx_nat")
                nc.gpsimd.dma_start(
                    qx_nat[:], p_qx[:].rearrange("(qt p) c -> p qt c", p=128)
                )
                for ct in range(NCT):
                    ps = evp.tile([128, 512], FP32, tag="ev")
                    for qt in range(NQT):
                        nc.tensor.matmul(
                            ps[:, qt * 128 : (qt + 1) * 128],
                            qx_nat[:, qt, ct * 128 : (ct + 1) * 128],
                            ident[:],
                            start=True,
                            stop=True,
                        )
                    sb = persist.tile([128, QS], BF16, tag=f"qxT_{ct}")
                    nc.vector.tensor_copy(sb[:], ps[:])
                    qxT.append(sb)

                kv_nat = stage.tile([128, NKT, C], BF16, tag="kv_nat")
                nc.gpsimd.dma_start(
                    kv_nat[:], p_kvx[:].rearrange("(kt p) c -> p kt c", p=128)
                )
                for ct in range(NCT):
                    sb = persist.tile([128, K], BF16, tag=f"kvT_{ct}")
                    for g in range(NKT // 4):  # 4 transposes -> one evac
                        ps = evp.tile([128, 512], FP32, tag="ev")
                        for i in range(4):
                            kt = g * 4 + i
                            nc.tensor.matmul(
                                ps[:, i * 128 : (i + 1) * 128],
                                kv_nat[:, kt, ct * 128 : (ct + 1) * 128],
                                ident[:],
                                start=True,
                                stop=True,
                            )
                        nc.vector.tensor_copy(
                            sb[:, g * 512 : (g + 1) * 512], ps[:]
                        )
                    kvT.append(sb)

                # ---- bias^T (bf16, via identity-matmul transposes) ----
                bias_nat = stage.tile([128, NQT, K], BF16, tag="bias_nat")
                nc.gpsimd.dma_start(
                    bias_nat[:], p_bias[:].rearrange("(qt p) k -> p qt k", p=128)
                )
                expBT = persist.tile([128, NKT, QS], BF16, tag="expBT")
                for kt in range(NKT):
                    ps = evp.tile([128, 512], FP32, tag="ev")
                    for qt in range(NQT):
                        nc.tensor.matmul(
                            ps[:, qt * 128 : (qt + 1) * 128],
                            bias_nat[:, qt, kt * 128 : (kt + 1) * 128],
                            ident[:],
                            start=True,
                            stop=True,
                        )
                    nc.scalar.activation(
                        expBT[:, kt, :], ps[:], mybir.ActivationFunctionType.Exp
                    )

                # ---- projections ----
                # kT[jt]: [128, K]  (partition j = (head % 4) * 32 + d)
                for jt in range(NCT):
                    sb = persist.tile([128, K], BF16, tag=f"kT_{jt}")
                    for tc_ in range(K // 512):
                        ps = evp.tile([128, 512], FP32, tag="ev")
                        for ct in range(NCT):
                            nc.tensor.matmul(
                                ps[:],
                                wt["Wk"][ct][:, jt * 128 : (jt + 1) * 128],
                                kvT[ct][:, tc_ * 512 : (tc_ + 1) * 512],
                                start=(ct == 0),
                                stop=(ct == NCT - 1),
                            )
                        nc.scalar.copy(sb[:, tc_ * 512 : (tc_ + 1) * 512], ps[:])
                    kT.append(sb)

                # qT[jt]: [128, QS] (pre-scaled by 1/sqrt(D) via ident_s)
                for jt in range(NCT):
                    ps = evp.tile([128, 512], FP32, tag="ev")
                    for ct in range(NCT):
                        nc.tensor.matmul(
                            ps[:],
                            wt["Wq"][ct][:, jt * 128 : (jt + 1) * 128],
                            qxT[ct][:],
                            start=(ct == 0),
                            stop=(ct == NCT - 1),
                        )
                    sb = persist.tile([128, QS], BF16, tag=f"qT_{jt}")
                    nc.scalar.copy(sb[:], ps[:])
                    qT.append(sb)

                # gate: tanh(0.5*x + 0.5*bg); sigmoid(x+bg) = 0.5*tanh + 0.5
                for jt in range(NCT):
                    ps = evp.tile([128, 512], FP32, tag="ev")
                    for ct in range(NCT):
                        nc.tensor.matmul(
                            ps[:],
                            wt["Wg"][ct][:, jt * 128 : (jt + 1) * 128],
                            qxT[ct][:],
                            start=(ct == 0),
                            stop=(ct == NCT - 1),
                        )
                    th = persist.tile([128, QS], BF16, tag=f"gtanh_{jt}")
                    nc.scalar.activation(
                        th[:],
                        ps[:],
                        mybir.ActivationFunctionType.Tanh,
                        bias=bg_half[:, jt : jt + 1],
                        scale=0.5,
                    )
                    g_half.append(th)

                # v[kt]: [128, C] natural layout (partition = key token)
                v_sb = persist.tile([128, NKT, C], BF16, tag="v_sb")
                for kt in range(NKT):
                    ps = evp.tile([128, 512], FP32, tag="ev")
                    for ct in range(NCT):
                        nc.tensor.matmul(
                            ps[:, :C],
                            kvT[ct][:, kt * 128 : (kt + 1) * 128],
                            wt["Wv"][ct][:],
                            start=(ct == 0),
                            stop=(ct == NCT - 1),
                        )
                    nc.scalar.copy(v_sb[:, kt, :], ps[:, :C])

            # ---- attention core ----
            ogT = []
            with (
                tc.tile_pool(name="acc", bufs=1, space="PSUM") as accp,
                tc.tile_pool(name="scores", bufs=1, space="PSUM") as scoresp,
                tc.tile_pool(name="expp", bufs=3) as expp,
            ):
                oT_ps = [
                    accp.tile([128, QS], FP32, tag=f"oT_{w}", name=f"oT_{w}") for w in range(2)
                ]
                sums_ps = [
                    accp.tile([128, QS], FP32, tag=f"sums_{w}", name=f"sums_{w}") for w in range(2)
                ]

                waves = [(kt, w) for kt in range(NKT) for w in range(2)]
                at_tiles = {}

                def emit_scores(i):
                    kt, w = waves[i]
                    sc = scoresp.tile([128, 4 * QS], FP32, tag="sc", name=f"sc_{i}")
                    # 16-tile K=32/M=32 grid: row = head, col = k-subtile
                    # (diagonal emission so LDWEIGHTS can pull ahead)
                    for dd in range(4):
                        for s in range(4):
                            ks = (s + dd) % 4
                            nc.tensor.matmul(
                                sc[32 * ks : 32 * (ks + 1), s * QS : (s + 1) * QS],
                                kT[w][
                                    32 * s : 32 * (s + 1),
                                    kt * 128 + 32 * ks : kt * 128 + 32 * (ks + 1),
                                ],
                                qT[w][32 * s : 32 * (s + 1), :],
                                start=True,
                                stop=True,
                                tile_position=(32 * s, 32 * ks),
                            )
                    ex = expp.tile([128, 4 * QS], BF16, tag="ex", name=f"ex_{i}")
                    nc.scalar.activation(
                        ex[:], sc[:], mybir.ActivationFunctionType.Exp
                    )
                    at = expp.tile([128, 4, QS], BF16, tag="at", name=f"at_{i}")
                    nc.vector.tensor_tensor(
                        out=at[:],
                        in0=ex[:].rearrange("p (h q) -> p h q", h=4),
                        in1=expBT[:, kt, :].unsqueeze(1).broadcast_to(
                            (128, 4, QS)
                        ),
                        op=mybir.AluOpType.mult,
                    )
                    at_tiles[i] = at

                def emit_o_sums(i):
                    kt, w = waves[i]
                    at = at_tiles.pop(i)
                    first, last = kt == 0, kt == NKT - 1
                    for s in range(4):
                        nc.tensor.matmul(
                            oT_ps[w][32 * s : 32 * (s + 1), :],
                            v_sb[:, kt, (4 * w + s) * D : (4 * w + s + 1) * D],
                            at[:, s, :],
                            start=first,
                            stop=last,
                            tile_position=(0, 32 * s),
                        )
                    for s in range(4):
                        nc.tensor.matmul(
                            sums_ps[w][32 * s : 32 * (s + 1), :],
                            ones_mat[:],
                            at[:, s, :],
                            start=first,
                            stop=last,
                            tile_position=(0, 32 * s),
                        )

                # depth-2 software pipeline: PE runs o/sums(i-2) then
                # scores(i); ACT exp(i) and DVE mul(i) hide underneath.
                for i in range(len(waves)):
                    if i >= 2:
                        emit_o_sums(i - 2)
                    emit_scores(i)
                emit_o_sums(len(waves) - 2)
                emit_o_sums(len(waves) - 1)

                # ---- normalize + gate:  og = oT * g * (1/Z) ----
                # sums_ps rows 32s..32s+32 all hold head (4w+s)'s Z[q].
                recipz = persist.tile([128, 2, QS], FP32, tag="recipz")
                for w in range(2):
                    nc.vector.reciprocal(recipz[:, w, :], sums_ps[w][:])
                for w in range(2):
                    zg = persist.tile([128, QS], BF16, tag=f"zg_{w}")
                    nc.vector.tensor_scalar(
                        zg[:],
                        g_half[w][:],
                        0.5,
                        0.5,
                        mybir.AluOpType.mult,
                        mybir.AluOpType.add,
                    )
                    nc.vector.tensor_mul(zg[:], zg[:], recipz[:, w, :])
                    og = persist.tile([128, QS], BF16, tag=f"ogT_{w}")
                    nc.vector.tensor_mul(og[:], oT_ps[w][:], zg[:])
                    ogT.append(og)

            # ---- output projection (natural layout) + bo ----
            out_sb = persist.tile([128, NQT, C], FP32, tag="out_sb")
            with tc.tile_pool(name="outp", bufs=2, space="PSUM") as outp:
                for qt in range(NQT):
                    ps = outp.tile([128, C], FP32, tag="outps")
                    for ct in range(NCT):
                        nc.tensor.matmul(
                            ps[:],
                            ogT[ct][:, qt * 128 : (qt + 1) * 128],
                            wt["Wo"][ct][:],
                            start=(ct == 0),
                            stop=False,
                        )
                    nc.tensor.matmul(
                        ps[:],
                        ones_row[:],
                        bo_row[:],
                        start=False,
                        stop=True,
                    )
                    nc.vector.tensor_copy(out_sb[:, qt, :], ps[:])

            nc.sync.dma_start(
                p_out[:].rearrange("(qt p) c -> p qt c", p=128), out_sb[:]
            )

    _split_multi_waits(nc)
    return nc


# ---------------------------------------------------------------------------


def _shard_inputs(inputs):
    """Full inputs -> per-core input maps."""
    in_maps = []
    for c in range(N_CORES):
        b, qc = divmod(c, 4)
        qs = qc * QS
        m = {
            "q_x": inputs["q_x"][b, qs : qs + QS, :],
            "kv_x": inputs["kv_x"][b],
            "bias": inputs["bias"][b, 0, qs : qs + QS, :],
            "Wq": inputs["Wq"],
            "Wk": inputs["Wk"],
            "Wv": inputs["Wv"],
            "Wo": inputs["Wo"],
            "bo": inputs["bo"],
            "Wg": inputs["Wg"],
            "bg": inputs["bg"],
        }
        m = {
            k: np.ascontiguousarray(np.asarray(v, dtype=np.float32))
            for k, v in m.items()
        }
        in_maps.append(m)
    return in_maps


def run(inputs, trace=False, tmpdir=None):
    """Run the kernel; returns (full_output, BassKernelResults)."""
    nc = build_graph()
    in_maps = _shard_inputs(inputs)
    res = run_bass_kernel_spmd(
        nc, in_maps, core_ids=list(range(N_CORES)), trace=trace, tmpdir=tmpdir
    )
    out = np.empty((B, Q, C), dtype=np.float32)
    for c in range(N_CORES):
        b, qc = divmod(c, 4)
        out[b, qc * QS : (qc + 1) * QS, :] = res.results[c]["out"]
    return out, res


def kernel(**inputs):
    out, _ = run(inputs, trace=False)
    return out
